# revision 10
# baseline (speedup 1.0000x reference)
"""AdaptiveFilterBank Trainium2 kernel (8 NeuronCores, data-parallel over batch).

Math: reference = conv1d(x, filters) then per-sample softmax-weighted sum over
the 8 filter channels. The weighted sum commutes with the (linear) conv, so
each sample needs ONE length-31 conv with a combined per-sample filter
    kb[b] = softmax(MLP(features[b])) @ filter_params      (tiny, host-side)

Device formulation (overlapped interleave, P=98): per sample lay x out as
    X[q, c] = x[c*98 + q - 15]      (zero-padded), [128, 1338] in SBUF
so each SBUF column holds a 128-wide window covering the 98 outputs of that
column plus the +-15 conv halo. Then the whole 'same' cross-correlation is ONE
matmul per output tile:
    Y[m, c] = sum_q T[q, m] X[q, c],   T[q, m] = kb[q - m]  (0 <= q-m <= 30)
with Y[m, c] = y[c*98 + m], m in [0, 98).

All PE traffic is bf16; accumulation fp32 in PSUM; HBM traffic bf16 both ways.

DMA schedule (v2): profile showed the v1 single-ring schedule left the SDMA
engines idle 52% of the span — every DMA shared the one qSPDynamicHW FIFO, so
each transfer's ~1-2us completion-receipt stall (sem-inc descriptor waits for
the data-write receipt) serialized with the next transfer's data, and output
writes interleaved with the input stream. v2 spreads traffic over all three
DGE rings:
  - SP   HWDGE ring: input chunks 0,2,4,6 (one per even sample)
  - ACT  HWDGE ring: tw (Toeplitz weights) first, then chunks 1,3,5,7
  - Pool SWDGE ring: all output-group DMAs (issued by the otherwise-idle Q7)
Alternating rings lets one ring's receipt stall hide behind the other ring's
data, and keeps output HBM-write receipts off the input stream entirely.
PE additionally runs 6 throwaway matmuls at program start so the HAM clock
gate (cold 1.2 GHz -> warm 2.4 GHz after ~3.4us of activity) is released by
the time the first real chunk lands.

Sharding: batch 64 -> 8 samples per core; filter/MLP params host-computed.
"""

import numpy as np

B = 64
L = 131072
N_CORES = 8
BPC = B // N_CORES          # samples per core
KLEN = 31
PAD = 15
P = 98                      # outputs per interleave column (128 - 30 halo)
NCOLS = 1338                # ceil(L / P) input/output columns per sample
NSPLIT = (512, 512, 314)    # matmul N tiling of the 1338 columns
_CACHE = {}


def _build_graph():
    """Raw Bacc graph with hand-rolled semaphores (Tile's fixed epilogue —
    kernel-tail drain + EVSEM butterfly — measured ~9 us, so we skip Tile)."""
    from concourse import bacc, mybir

    dt = mybir.dt
    nc = bacc.Bacc("TRN2", target_bir_lowering=False, debug=False,
                   num_devices=N_CORES)

    x_ext = nc.dram_tensor("xt", [128, BPC * NCOLS], dt.bfloat16,
                           kind="ExternalInput").ap()
    t_ext = nc.dram_tensor("tw", [128, BPC * 128], dt.bfloat16,
                           kind="ExternalInput").ap()
    out_ext = nc.dram_tensor("out", [P, BPC * NCOLS], dt.bfloat16,
                             kind="ExternalOutput").ap()

    NOT = 8                 # output staging slots (1 per sample: no WAR waits)
    c0s = [sum(NSPLIT[:h]) for h in range(len(NSPLIT))]
    # engine that copies tile (b, h): ACT for the middle tile of every sample
    # plus the short tile of the last three samples (load balance)
    def is_act(b, h):
        return h == 1 or (h == 2 and b >= BPC - 3)

    def copies_done_before(k):
        """(#DVE, #ACT) copies among global tiles 0..k-1."""
        nv = sum(1 for j in range(k)
                 if not is_act(j // len(NSPLIT), j % len(NSPLIT)))
        ns = k - nv
        return nv, ns

    from contextlib import ExitStack
    stack = ExitStack()
    with (
        nc.sbuf_tensor("xt_sb", [128, BPC * NCOLS], dt.bfloat16) as xt_sb,
        nc.sbuf_tensor("tw_sb", [128, BPC * 128], dt.bfloat16) as tw_sb,
        nc.sbuf_tensor("ot_sb", [P, NOT * NCOLS], dt.bfloat16) as ot_sb,
        nc.psum_tensor("ps", [128, 8 * 512], dt.float32) as ps,
        nc.semaphore("s_tw") as s_tw,
        nc.semaphore("s_mm") as s_mm,
        nc.semaphore("s_cv") as s_cv,
        nc.semaphore("s_cs") as s_cs,
        stack,
    ):
        # one sem per input chunk: a sem shared by several DMAs can't prove
        # WHICH one finished (engines complete out of lockstep). Sample 0's
        # chunk is split in half so the first matmul can start ~0.5us sooner.
        s_ch = [stack.enter_context(nc.semaphore(f"s_ch{i}"))
                for i in range(BPC)]
        s_c0a = stack.enter_context(nc.semaphore("s_c0a"))
        # outputs: per-sample DMAs alternating Pool-SWDGE (even) / SP-HWDGE
        # (odd) rings, one shared sem per ring. A partial count can't prove a
        # PREFIX of DMAs done (engines complete out of lockstep), but the
        # FULL sum (16 per DMA x 4 DMAs = 64) does prove all four landed.
        s_prA = stack.enter_context(nc.semaphore("s_prA"))   # Pool ring
        s_prB = stack.enter_context(nc.semaphore("s_prB"))   # SP ring
        HALF0 = 669             # columns of sample 0 in its first half-chunk
        block_cm = nc.Block(no_gpsimd_drain=True)
        block = block_cm.__enter__()

        def out_dma(eng, b, sem):
            so = (b % NOT) * NCOLS
            eng.dma_start(out=out_ext[:, b * NCOLS:(b + 1) * NCOLS],
                          in_=ot_sb[:, so:so + NCOLS]).then_inc(sem, 16)

        @block.sync
        def _(sync):
            # weights first (gate every matmul), then even chunks; the SP
            # ring's first packet hits ~1.4us after dispatch vs ~2.8us on the
            # ACT ring (ACT_TABLE_LOAD runs there first)
            sync.dma_start(out=tw_sb[:], in_=t_ext[:]).then_inc(s_tw, 16)
            sync.dma_start(out=xt_sb[:, 0:HALF0],
                           in_=x_ext[:, 0:HALF0]).then_inc(s_c0a, 16)
            sync.dma_start(out=xt_sb[:, HALF0:NCOLS],
                           in_=x_ext[:, HALF0:NCOLS]).then_inc(s_ch[0], 16)
            for b in range(2, BPC, 2):
                sync.dma_start(
                    out=xt_sb[:, b * NCOLS:(b + 1) * NCOLS],
                    in_=x_ext[:, b * NCOLS:(b + 1) * NCOLS],
                ).then_inc(s_ch[b], 16)
            # odd samples' outputs ride this (now idle) ring; sample 7 is the
            # kernel tail so it gets the fast HWDGE path
            for b in range(1, BPC, 2):
                nv, ns = copies_done_before(len(NSPLIT) * (b + 1))
                sync.wait_ge(s_cv, nv)
                sync.wait_ge(s_cs, ns)
                out_dma(sync, b, s_prB)

        @block.tensor
        def _(tensor):
            # HAM pre-warm: garbage matmuls into bank 7 (overwritten later by
            # real tile k=7 whose readers gate on s_mm) while inputs stream in
            for _ in range(6):
                tensor.matmul(ps[:, 7 * 512:7 * 512 + 512],
                              tw_sb[:, 0:128], xt_sb[:, 0:512],
                              start=True, stop=True)
            tensor.wait_ge(s_tw, 16)
            for b in range(BPC):
                if b == 0:
                    tensor.wait_ge(s_c0a, 16)    # covers tile h=0 (cols<669)
                else:
                    tensor.wait_ge(s_ch[b], 16)
                for h, n in enumerate(NSPLIT):
                    if b == 0 and h == 1:
                        tensor.wait_ge(s_ch[0], 16)   # rest of sample 0
                    k = len(NSPLIT) * b + h
                    if k >= 8:
                        # recycled bank is drained by exactly ONE engine
                        nv, ns = copies_done_before(k - 7)
                        if is_act((k - 8) // len(NSPLIT), (k - 8) % len(NSPLIT)):
                            tensor.wait_ge(s_cs, ns)
                        else:
                            tensor.wait_ge(s_cv, nv)
                    bank = (k % 8) * 512
                    c0 = c0s[h]
                    tensor.matmul(
                        ps[:, bank:bank + n],
                        tw_sb[:, b * 128:(b + 1) * 128],
                        xt_sb[:, b * NCOLS + c0:b * NCOLS + c0 + n],
                        start=True, stop=True).then_inc(s_mm, 1)

        @block.vector
        def _(vector):
            for b in range(BPC):
                so = (b % NOT) * NCOLS
                for h, n in enumerate(NSPLIT):
                    if is_act(b, h):
                        continue
                    k = len(NSPLIT) * b + h
                    vector.wait_ge(s_mm, k + 1)
                    bank = (k % 8) * 512
                    c0 = c0s[h]
                    vector.tensor_copy(ot_sb[:, so + c0:so + c0 + n],
                                       ps[0:P, bank:bank + n]).then_inc(s_cv, 1)

        @block.scalar
        def _(scalar):
            # odd chunks on the ACT HWDGE ring
            for b in range(1, BPC, 2):
                scalar.dma_start(
                    out=xt_sb[:, b * NCOLS:(b + 1) * NCOLS],
                    in_=x_ext[:, b * NCOLS:(b + 1) * NCOLS],
                ).then_inc(s_ch[b], 16)
            for b in range(BPC):
                so = (b % NOT) * NCOLS
                for h, n in enumerate(NSPLIT):
                    if not is_act(b, h):
                        continue
                    k = len(NSPLIT) * b + h
                    scalar.wait_ge(s_mm, k + 1)
                    bank = (k % 8) * 512
                    c0 = c0s[h]
                    scalar.copy(ot_sb[:, so + c0:so + c0 + n],
                                ps[0:P, bank:bank + n]).then_inc(s_cs, 1)

        @block.gpsimd
        def _(gpsimd):
            # even samples' outputs on the SWDGE ring: receipt stalls stay
            # off the input rings, and Q7 is idle anyway
            for b in range(0, BPC, 2):
                nv, ns = copies_done_before(len(NSPLIT) * (b + 1))
                gpsimd.wait_ge(s_cv, nv)
                gpsimd.wait_ge(s_cs, ns)
                out_dma(gpsimd, b, s_prA)
            gpsimd.wait_ge(s_prA, 16 * (BPC // 2))
            gpsimd.wait_ge(s_prB, 16 * (BPC // 2))

        # block exit emits drain + all-engine barrier; then reset the kernel
        # sems to 0 so the NEFF can re-execute
        block_cm.__exit__(None, None, None)
        nums = sorted(s.num for s in
                      [s_tw, s_mm, s_cv, s_cs, s_c0a, s_prA, s_prB] + s_ch)
        nc.gpsimd.dma_reset(range(nums[0], nums[-1] + 1))
        nc.gpsimd.sem_clear(range(nums[0], nums[-1] + 1))

    nc.compile()
    return nc


def _get_graph():
    if "nc" not in _CACHE:
        _CACHE["nc"] = _build_graph()
    return _CACHE["nc"]


def _host_prep(x, features, filter_params, W1, b1, W2, b2):
    """Selector MLP + combined filters + layout prep. All tiny or memory-bound."""
    import ml_dtypes
    from numpy.lib.stride_tricks import sliding_window_view
    bf16 = ml_dtypes.bfloat16

    x = np.ascontiguousarray(x, dtype=np.float32)
    # selector MLP (torch Linear convention)
    h = np.maximum(features @ W1.T + b1, 0.0)
    logits = h @ W2.T + b2
    e = np.exp(logits - logits.max(axis=-1, keepdims=True))
    w = e / e.sum(axis=-1, keepdims=True)                      # (B, 8)
    kb = (w @ filter_params[:, 0, :]).astype(np.float32)       # (B, 31)

    # overlapped interleave: X[b, q, c] = x[b, c*98 + q - 15]
    span = (NCOLS - 1) * P + 128
    xp = np.zeros((B, span), dtype=np.float32)
    xp[:, PAD:PAD + L] = x
    win = sliding_window_view(xp, 128, axis=1)                 # (B, span-127, 128)
    xt = win[:, ::P][:, :NCOLS].transpose(0, 2, 1)             # (B, 128, 1338)

    # banded Toeplitz weight: T[q, m] = kb[q - m], 0 <= q-m <= 30
    q = np.arange(128)[:, None]
    m = np.arange(128)[None, :]          # padded to 128 cols for PE FWL
    t_i = q - m
    mask = (t_i >= 0) & (t_i <= 30)
    tw = np.zeros((B, 128, 128), dtype=np.float32)
    tw[:, mask] = kb[:, t_i[mask]]

    def pack(a):  # (B, Pdim, C) -> per-core (Pdim, BPC*C) bf16
        Pd, C = a.shape[1], a.shape[2]
        return [np.ascontiguousarray(
                    a[i * BPC:(i + 1) * BPC].transpose(1, 0, 2).reshape(Pd, BPC * C)
                ).astype(bf16) for i in range(N_CORES)]

    return pack(xt), pack(tw)


def _run(inputs, trace=False, trace_cores=None):
    """Shard, execute on 8 NeuronCores, gather. Returns (y, exec_time_ns)."""
    from concourse.bass_utils import run_bass_kernel_spmd

    xts, tws = _host_prep(**inputs)
    nc = _get_graph()
    in_maps = [{"xt": xts[i], "tw": tws[i]} for i in range(N_CORES)]
    res = run_bass_kernel_spmd(nc, in_maps, core_ids=list(range(N_CORES)),
                               trace=trace, trace_cores=trace_cores)
    # gather: per-core out [P, BPC*NCOLS]; sample block.T.flatten()[:L] -> y[b]
    y = np.empty((B, L), dtype=np.float32)
    for i in range(N_CORES):
        yc = np.asarray(res.results[i]["out"]).astype(np.float32)
        yc = yc.reshape(P, BPC, NCOLS).transpose(1, 2, 0)      # (BPC, NCOLS, P)
        y[i * BPC:(i + 1) * BPC] = yc.reshape(BPC, NCOLS * P)[:, :L]
    return y, res.exec_time_ns


def kernel(x, features, filter_params, W1, b1, W2, b2):
    y, _ = _run(dict(x=x, features=features, filter_params=filter_params,
                     W1=W1, b1=b1, W2=W2, b2=b2))
    return y


# revision 14
# speedup vs baseline: 1.0362x; 1.0362x over previous
"""AdaptiveFilterBank Trainium2 kernel (8 NeuronCores, data-parallel over batch).

Math: reference = conv1d(x, filters) then per-sample softmax-weighted sum over
the 8 filter channels. The weighted sum commutes with the (linear) conv, so
each sample needs ONE length-31 conv with a combined per-sample filter
    kb[b] = softmax(MLP(features[b])) @ filter_params      (tiny, host-side)

Device formulation (overlapped interleave, P=98): per sample lay x out as
    X[q, c] = x[c*98 + q - 15]      (zero-padded), [128, 1338] in SBUF
so each SBUF column holds a 128-wide window covering the 98 outputs of that
column plus the +-15 conv halo. Then the whole 'same' cross-correlation is ONE
matmul per output tile:
    Y[m, c] = sum_q T[q, m] X[q, c],   T[q, m] = kb[q - m]  (0 <= q-m <= 30)
with Y[m, c] = y[c*98 + m], m in [0, 98).

All PE traffic is bf16; accumulation fp32 in PSUM; HBM traffic bf16 both ways.

DMA schedule (v2): profile showed the v1 single-ring schedule left the SDMA
engines idle 52% of the span — every DMA shared the one qSPDynamicHW FIFO, so
each transfer's ~1-2us completion-receipt stall (sem-inc descriptor waits for
the data-write receipt) serialized with the next transfer's data, and output
writes interleaved with the input stream. v2 spreads traffic over all three
DGE rings:
  - SP   HWDGE ring: input chunks 0,2,4,6 (one per even sample)
  - ACT  HWDGE ring: tw (Toeplitz weights) first, then chunks 1,3,5,7
  - Pool SWDGE ring: all output-group DMAs (issued by the otherwise-idle Q7)
Alternating rings lets one ring's receipt stall hide behind the other ring's
data, and keeps output HBM-write receipts off the input stream entirely.
PE additionally runs 6 throwaway matmuls at program start so the HAM clock
gate (cold 1.2 GHz -> warm 2.4 GHz after ~3.4us of activity) is released by
the time the first real chunk lands.

Sharding: batch 64 -> 8 samples per core; filter/MLP params host-computed.
"""

import numpy as np

B = 64
L = 131072
N_CORES = 8
BPC = B // N_CORES          # samples per core
KLEN = 31
PAD = 15
P = 98                      # outputs per interleave column (128 - 30 halo)
NCOLS = 1338                # ceil(L / P) input/output columns per sample
NSPLIT = (512, 512, 314)    # matmul N tiling of the 1338 columns
_CACHE = {}


def _build_graph():
    """Raw Bacc graph with hand-rolled semaphores (Tile's fixed epilogue —
    kernel-tail drain + EVSEM butterfly — measured ~9 us, so we skip Tile)."""
    from concourse import bacc, mybir

    dt = mybir.dt
    nc = bacc.Bacc("TRN2", target_bir_lowering=False, debug=False,
                   num_devices=N_CORES)

    x_ext = nc.dram_tensor("xt", [128, BPC * NCOLS], dt.bfloat16,
                           kind="ExternalInput").ap()
    t_ext = nc.dram_tensor("tw", [128, BPC * 128], dt.bfloat16,
                           kind="ExternalInput").ap()
    out_ext = nc.dram_tensor("out", [P, BPC * NCOLS], dt.bfloat16,
                             kind="ExternalOutput").ap()

    NOT = 8                 # output staging slots (1 per sample: no WAR waits)
    c0s = [sum(NSPLIT[:h]) for h in range(len(NSPLIT))]
    # engine that copies tile (b, h): ACT for the middle tile of every sample
    # plus the short tile of the last three samples (load balance)
    def is_act(b, h):
        return h == 1 or (h == 2 and b >= BPC - 3)

    def copies_done_before(k):
        """(#DVE, #ACT) copies among global tiles 0..k-1."""
        nv = sum(1 for j in range(k)
                 if not is_act(j // len(NSPLIT), j % len(NSPLIT)))
        ns = k - nv
        return nv, ns

    from contextlib import ExitStack
    stack = ExitStack()
    with (
        nc.sbuf_tensor("xt_sb", [128, BPC * NCOLS], dt.bfloat16) as xt_sb,
        nc.sbuf_tensor("tw_sb", [128, BPC * 128], dt.bfloat16) as tw_sb,
        nc.sbuf_tensor("ot_sb", [P, NOT * NCOLS], dt.bfloat16) as ot_sb,
        nc.psum_tensor("ps", [128, 8 * 512], dt.float32) as ps,
        nc.semaphore("s_tw") as s_tw,
        nc.semaphore("s_mm") as s_mm,
        nc.semaphore("s_cv") as s_cv,
        nc.semaphore("s_cs") as s_cs,
        stack,
    ):
        # one sem per input chunk: a sem shared by several DMAs can't prove
        # WHICH one finished (engines complete out of lockstep)
        s_ch = [stack.enter_context(nc.semaphore(f"s_ch{i}"))
                for i in range(BPC)]
        # outputs: per-sample DMAs spread over ALL THREE rings (one output
        # issues per ~2.3us per ring — data + HBM-write receipt — so one or
        # two rings can't keep up with the ~1.1us/sample drain cadence). One
        # shared sem per ring: a partial count can't prove a PREFIX of DMAs
        # done, but the FULL sum (16 per DMA) does prove all of them landed.
        OUT_RING = {0: 'A', 3: 'A', 6: 'A',     # Pool SWDGE
                    1: 'B', 4: 'B',             # SP  HWDGE
                    2: 'C', 5: 'C', 7: 'C'}     # ACT HWDGE (idle after
                                                # inputs; its own drains gate
                                                # the tail sample anyway)
        s_prA = stack.enter_context(nc.semaphore("s_prA"))
        s_prB = stack.enter_context(nc.semaphore("s_prB"))
        s_prC = stack.enter_context(nc.semaphore("s_prC"))
        s_pr = {'A': s_prA, 'B': s_prB, 'C': s_prC}
        block_cm = nc.Block(no_gpsimd_drain=True)
        block = block_cm.__enter__()

        def out_dma(eng, b):
            so = (b % NOT) * NCOLS
            eng.dma_start(out=out_ext[:, b * NCOLS:(b + 1) * NCOLS],
                          in_=ot_sb[:, so:so + NCOLS]
                          ).then_inc(s_pr[OUT_RING[b]], 16)

        @block.sync
        def _(sync):
            # weights first (gate every matmul), then even chunks; the SP
            # ring's first packet hits ~1.4us after dispatch vs ~2.8us on the
            # ACT ring (ACT_TABLE_LOAD runs there first)
            sync.dma_start(out=tw_sb[:], in_=t_ext[:]).then_inc(s_tw, 16)
            for b in range(0, BPC, 2):
                sync.dma_start(
                    out=xt_sb[:, b * NCOLS:(b + 1) * NCOLS],
                    in_=x_ext[:, b * NCOLS:(b + 1) * NCOLS],
                ).then_inc(s_ch[b], 16)
            for b in sorted(k for k, r in OUT_RING.items() if r == 'B'):
                nv, ns = copies_done_before(len(NSPLIT) * (b + 1))
                sync.wait_ge(s_cv, nv)
                sync.wait_ge(s_cs, ns)
                out_dma(sync, b)

        @block.tensor
        def _(tensor):
            # HAM pre-warm: garbage matmuls into bank 7 (overwritten later by
            # real tile k=7 whose readers gate on s_mm) while inputs stream in
            for _ in range(6):
                tensor.matmul(ps[:, 7 * 512:7 * 512 + 512],
                              tw_sb[:, 0:128], xt_sb[:, 0:512],
                              start=True, stop=True)
            tensor.wait_ge(s_tw, 16)
            for b in range(BPC):
                tensor.wait_ge(s_ch[b], 16)
                for h, n in enumerate(NSPLIT):
                    k = len(NSPLIT) * b + h
                    if k >= 8:
                        # recycled bank is drained by exactly ONE engine
                        nv, ns = copies_done_before(k - 7)
                        if is_act((k - 8) // len(NSPLIT), (k - 8) % len(NSPLIT)):
                            tensor.wait_ge(s_cs, ns)
                        else:
                            tensor.wait_ge(s_cv, nv)
                    bank = (k % 8) * 512
                    c0 = c0s[h]
                    tensor.matmul(
                        ps[:, bank:bank + n],
                        tw_sb[:, b * 128:(b + 1) * 128],
                        xt_sb[:, b * NCOLS + c0:b * NCOLS + c0 + n],
                        start=True, stop=True).then_inc(s_mm, 1)

        @block.vector
        def _(vector):
            for b in range(BPC):
                so = (b % NOT) * NCOLS
                for h, n in enumerate(NSPLIT):
                    if is_act(b, h):
                        continue
                    k = len(NSPLIT) * b + h
                    vector.wait_ge(s_mm, k + 1)
                    bank = (k % 8) * 512
                    c0 = c0s[h]
                    vector.tensor_copy(ot_sb[:, so + c0:so + c0 + n],
                                       ps[0:P, bank:bank + n]).then_inc(s_cv, 1)

        @block.scalar
        def _(scalar):
            # odd chunks on the ACT HWDGE ring
            for b in range(1, BPC, 2):
                scalar.dma_start(
                    out=xt_sb[:, b * NCOLS:(b + 1) * NCOLS],
                    in_=x_ext[:, b * NCOLS:(b + 1) * NCOLS],
                ).then_inc(s_ch[b], 16)
            for b in range(BPC):
                so = (b % NOT) * NCOLS
                for h, n in enumerate(NSPLIT):
                    if not is_act(b, h):
                        continue
                    k = len(NSPLIT) * b + h
                    scalar.wait_ge(s_mm, k + 1)
                    bank = (k % 8) * 512
                    c0 = c0s[h]
                    scalar.copy(ot_sb[:, so + c0:so + c0 + n],
                                ps[0:P, bank:bank + n]).then_inc(s_cs, 1)
                # this ring's outputs issue right after the drains they need
                if OUT_RING.get(b) == 'C':
                    nv, _ = copies_done_before(len(NSPLIT) * (b + 1))
                    scalar.wait_ge(s_cv, nv)
                    out_dma(scalar, b)

        @block.gpsimd
        def _(gpsimd):
            # this ring's outputs ride SWDGE: receipt stalls stay off the
            # input rings, and Q7 is idle anyway
            for b in sorted(k for k, r in OUT_RING.items() if r == 'A'):
                nv, ns = copies_done_before(len(NSPLIT) * (b + 1))
                gpsimd.wait_ge(s_cv, nv)
                gpsimd.wait_ge(s_cs, ns)
                out_dma(gpsimd, b)
            for ring, sem in s_pr.items():
                n = sum(1 for r in OUT_RING.values() if r == ring)
                gpsimd.wait_ge(sem, 16 * n)

        # block exit emits drain + all-engine barrier; then reset the kernel
        # sems to 0 so the NEFF can re-execute
        block_cm.__exit__(None, None, None)
        nums = sorted(s.num for s in
                      [s_tw, s_mm, s_cv, s_cs, s_prA, s_prB, s_prC] + s_ch)
        nc.gpsimd.dma_reset(range(nums[0], nums[-1] + 1))
        nc.gpsimd.sem_clear(range(nums[0], nums[-1] + 1))

    nc.compile()
    return nc


def _get_graph():
    if "nc" not in _CACHE:
        _CACHE["nc"] = _build_graph()
    return _CACHE["nc"]


def _host_prep(x, features, filter_params, W1, b1, W2, b2):
    """Selector MLP + combined filters + layout prep. All tiny or memory-bound."""
    import ml_dtypes
    from numpy.lib.stride_tricks import sliding_window_view
    bf16 = ml_dtypes.bfloat16

    x = np.ascontiguousarray(x, dtype=np.float32)
    # selector MLP (torch Linear convention)
    h = np.maximum(features @ W1.T + b1, 0.0)
    logits = h @ W2.T + b2
    e = np.exp(logits - logits.max(axis=-1, keepdims=True))
    w = e / e.sum(axis=-1, keepdims=True)                      # (B, 8)
    kb = (w @ filter_params[:, 0, :]).astype(np.float32)       # (B, 31)

    # overlapped interleave: X[b, q, c] = x[b, c*98 + q - 15]
    span = (NCOLS - 1) * P + 128
    xp = np.zeros((B, span), dtype=np.float32)
    xp[:, PAD:PAD + L] = x
    win = sliding_window_view(xp, 128, axis=1)                 # (B, span-127, 128)
    xt = win[:, ::P][:, :NCOLS].transpose(0, 2, 1)             # (B, 128, 1338)

    # banded Toeplitz weight: T[q, m] = kb[q - m], 0 <= q-m <= 30
    q = np.arange(128)[:, None]
    m = np.arange(128)[None, :]          # padded to 128 cols for PE FWL
    t_i = q - m
    mask = (t_i >= 0) & (t_i <= 30)
    tw = np.zeros((B, 128, 128), dtype=np.float32)
    tw[:, mask] = kb[:, t_i[mask]]

    def pack(a):  # (B, Pdim, C) -> per-core (Pdim, BPC*C) bf16
        Pd, C = a.shape[1], a.shape[2]
        return [np.ascontiguousarray(
                    a[i * BPC:(i + 1) * BPC].transpose(1, 0, 2).reshape(Pd, BPC * C)
                ).astype(bf16) for i in range(N_CORES)]

    return pack(xt), pack(tw)


def _run(inputs, trace=False, trace_cores=None):
    """Shard, execute on 8 NeuronCores, gather. Returns (y, exec_time_ns)."""
    from concourse.bass_utils import run_bass_kernel_spmd

    xts, tws = _host_prep(**inputs)
    nc = _get_graph()
    in_maps = [{"xt": xts[i], "tw": tws[i]} for i in range(N_CORES)]
    res = run_bass_kernel_spmd(nc, in_maps, core_ids=list(range(N_CORES)),
                               trace=trace, trace_cores=trace_cores)
    # gather: per-core out [P, BPC*NCOLS]; sample block.T.flatten()[:L] -> y[b]
    y = np.empty((B, L), dtype=np.float32)
    for i in range(N_CORES):
        yc = np.asarray(res.results[i]["out"]).astype(np.float32)
        yc = yc.reshape(P, BPC, NCOLS).transpose(1, 2, 0)      # (BPC, NCOLS, P)
        y[i * BPC:(i + 1) * BPC] = yc.reshape(BPC, NCOLS * P)[:, :L]
    return y, res.exec_time_ns


def kernel(x, features, filter_params, W1, b1, W2, b2):
    y, _ = _run(dict(x=x, features=features, filter_params=filter_params,
                     W1=W1, b1=b1, W2=W2, b2=b2))
    return y


# revision 16
# speedup vs baseline: 1.0367x; 1.0005x over previous
"""AdaptiveFilterBank Trainium2 kernel (8 NeuronCores, data-parallel over batch).

Math: reference = conv1d(x, filters) then per-sample softmax-weighted sum over
the 8 filter channels. The weighted sum commutes with the (linear) conv, so
each sample needs ONE length-31 conv with a combined per-sample filter
    kb[b] = softmax(MLP(features[b])) @ filter_params      (tiny, host-side)

Device formulation (overlapped interleave, P=98): per sample lay x out as
    X[q, c] = x[c*98 + q - 15]      (zero-padded), [128, 1338] in SBUF
so each SBUF column holds a 128-wide window covering the 98 outputs of that
column plus the +-15 conv halo. Then the whole 'same' cross-correlation is ONE
matmul per output tile:
    Y[m, c] = sum_q T[q, m] X[q, c],   T[q, m] = kb[q - m]  (0 <= q-m <= 30)
with Y[m, c] = y[c*98 + m], m in [0, 98).

All PE traffic is bf16; accumulation fp32 in PSUM; HBM traffic bf16 both ways.

DMA schedule (v2): profile showed the v1 single-ring schedule left the SDMA
engines idle 52% of the span — every DMA shared the one qSPDynamicHW FIFO, so
each transfer's ~1-2us completion-receipt stall (sem-inc descriptor waits for
the data-write receipt) serialized with the next transfer's data, and output
writes interleaved with the input stream. v2 spreads traffic over all three
DGE rings:
  - SP   HWDGE ring: input chunks 0,2,4,6 (one per even sample)
  - ACT  HWDGE ring: tw (Toeplitz weights) first, then chunks 1,3,5,7
  - Pool SWDGE ring: all output-group DMAs (issued by the otherwise-idle Q7)
Alternating rings lets one ring's receipt stall hide behind the other ring's
data, and keeps output HBM-write receipts off the input stream entirely.
PE additionally runs 6 throwaway matmuls at program start so the HAM clock
gate (cold 1.2 GHz -> warm 2.4 GHz after ~3.4us of activity) is released by
the time the first real chunk lands.

Sharding: batch 64 -> 8 samples per core; filter/MLP params host-computed.
"""

import numpy as np

B = 64
L = 131072
N_CORES = 8
BPC = B // N_CORES          # samples per core
KLEN = 31
PAD = 15
P = 98                      # outputs per interleave column (128 - 30 halo)
NCOLS = 1338                # ceil(L / P) input/output columns per sample
NSPLIT = (512, 512, 314)    # matmul N tiling of the 1338 columns
_CACHE = {}


def _build_graph():
    """Raw Bacc graph with hand-rolled semaphores (Tile's fixed epilogue —
    kernel-tail drain + EVSEM butterfly — measured ~9 us, so we skip Tile)."""
    from concourse import bacc, mybir

    dt = mybir.dt
    nc = bacc.Bacc("TRN2", target_bir_lowering=False, debug=False,
                   num_devices=N_CORES)

    x_ext = nc.dram_tensor("xt", [128, BPC * NCOLS], dt.bfloat16,
                           kind="ExternalInput").ap()
    t_ext = nc.dram_tensor("tw", [128, BPC * 128], dt.bfloat16,
                           kind="ExternalInput").ap()
    out_ext = nc.dram_tensor("out", [P, BPC * NCOLS], dt.bfloat16,
                             kind="ExternalOutput").ap()

    NOT = 8                 # output staging slots (1 per sample: no WAR waits)
    c0s = [sum(NSPLIT[:h]) for h in range(len(NSPLIT))]
    # engine that copies tile (b, h): ACT the middle tile of every sample;
    # for the LAST sample ACT takes h0+h1 and DVE only the short h2, so the
    # final drain (which gates the tail output DMA) lands ~1.7us sooner
    def is_act(b, h):
        if b == BPC - 1:
            return h in (0, 1)
        return h == 1

    def copies_done_before(k):
        """(#DVE, #ACT) copies among global tiles 0..k-1."""
        nv = sum(1 for j in range(k)
                 if not is_act(j // len(NSPLIT), j % len(NSPLIT)))
        ns = k - nv
        return nv, ns

    from contextlib import ExitStack
    stack = ExitStack()
    with (
        nc.sbuf_tensor("xt_sb", [128, BPC * NCOLS], dt.bfloat16) as xt_sb,
        nc.sbuf_tensor("tw_sb", [128, BPC * 128], dt.bfloat16) as tw_sb,
        nc.sbuf_tensor("ot_sb", [P, NOT * NCOLS], dt.bfloat16) as ot_sb,
        nc.psum_tensor("ps", [128, 8 * 512], dt.float32) as ps,
        nc.semaphore("s_tw") as s_tw,
        nc.semaphore("s_mm") as s_mm,
        nc.semaphore("s_cv") as s_cv,
        nc.semaphore("s_cs") as s_cs,
        stack,
    ):
        # one sem per input chunk: a sem shared by several DMAs can't prove
        # WHICH one finished (engines complete out of lockstep)
        s_ch = [stack.enter_context(nc.semaphore(f"s_ch{i}"))
                for i in range(BPC)]
        # outputs: per-sample DMAs spread over ALL THREE rings (one output
        # issues per ~2.3us per ring — data + HBM-write receipt — so one or
        # two rings can't keep up with the ~1.1us/sample drain cadence). One
        # shared sem per ring: a partial count can't prove a PREFIX of DMAs
        # done, but the FULL sum (16 per DMA) does prove all of them landed.
        OUT_RING = {0: 'A', 2: 'A',             # Pool SWDGE (slow ~80 GB/s:
                                                # early samples only)
                    1: 'B', 4: 'B', 6: 'B',     # SP  HWDGE
                    3: 'C', 5: 'C', 7: 'C'}     # ACT HWDGE (its own drains
                                                # gate the tail sample anyway)
        s_prA = stack.enter_context(nc.semaphore("s_prA"))
        s_prB = stack.enter_context(nc.semaphore("s_prB"))
        s_prC = stack.enter_context(nc.semaphore("s_prC"))
        s_pr = {'A': s_prA, 'B': s_prB, 'C': s_prC}
        block_cm = nc.Block(no_gpsimd_drain=True)
        block = block_cm.__enter__()

        def out_dma(eng, b):
            so = (b % NOT) * NCOLS
            eng.dma_start(out=out_ext[:, b * NCOLS:(b + 1) * NCOLS],
                          in_=ot_sb[:, so:so + NCOLS]
                          ).then_inc(s_pr[OUT_RING[b]], 16)

        @block.sync
        def _(sync):
            # weights first (gate every matmul), then even chunks; the SP
            # ring's first packet hits ~1.4us after dispatch vs ~2.8us on the
            # ACT ring (ACT_TABLE_LOAD runs there first)
            sync.dma_start(out=tw_sb[:], in_=t_ext[:]).then_inc(s_tw, 16)
            for b in range(0, BPC, 2):
                sync.dma_start(
                    out=xt_sb[:, b * NCOLS:(b + 1) * NCOLS],
                    in_=x_ext[:, b * NCOLS:(b + 1) * NCOLS],
                ).then_inc(s_ch[b], 16)
            for b in sorted(k for k, r in OUT_RING.items() if r == 'B'):
                nv, ns = copies_done_before(len(NSPLIT) * (b + 1))
                sync.wait_ge(s_cv, nv)
                sync.wait_ge(s_cs, ns)
                out_dma(sync, b)

        @block.tensor
        def _(tensor):
            # HAM pre-warm: garbage matmuls into bank 7 (overwritten later by
            # real tile k=7 whose readers gate on s_mm) while inputs stream in
            for _ in range(6):
                tensor.matmul(ps[:, 7 * 512:7 * 512 + 512],
                              tw_sb[:, 0:128], xt_sb[:, 0:512],
                              start=True, stop=True)
            tensor.wait_ge(s_tw, 16)
            for b in range(BPC):
                tensor.wait_ge(s_ch[b], 16)
                for h, n in enumerate(NSPLIT):
                    k = len(NSPLIT) * b + h
                    if k >= 8:
                        # recycled bank is drained by exactly ONE engine
                        nv, ns = copies_done_before(k - 7)
                        if is_act((k - 8) // len(NSPLIT), (k - 8) % len(NSPLIT)):
                            tensor.wait_ge(s_cs, ns)
                        else:
                            tensor.wait_ge(s_cv, nv)
                    bank = (k % 8) * 512
                    c0 = c0s[h]
                    tensor.matmul(
                        ps[:, bank:bank + n],
                        tw_sb[:, b * 128:(b + 1) * 128],
                        xt_sb[:, b * NCOLS + c0:b * NCOLS + c0 + n],
                        start=True, stop=True).then_inc(s_mm, 1)

        @block.vector
        def _(vector):
            for b in range(BPC):
                so = (b % NOT) * NCOLS
                for h, n in enumerate(NSPLIT):
                    if is_act(b, h):
                        continue
                    k = len(NSPLIT) * b + h
                    vector.wait_ge(s_mm, k + 1)
                    bank = (k % 8) * 512
                    c0 = c0s[h]
                    vector.tensor_copy(ot_sb[:, so + c0:so + c0 + n],
                                       ps[0:P, bank:bank + n]).then_inc(s_cv, 1)

        @block.scalar
        def _(scalar):
            # odd chunks on the ACT HWDGE ring
            for b in range(1, BPC, 2):
                scalar.dma_start(
                    out=xt_sb[:, b * NCOLS:(b + 1) * NCOLS],
                    in_=x_ext[:, b * NCOLS:(b + 1) * NCOLS],
                ).then_inc(s_ch[b], 16)
            for b in range(BPC):
                so = (b % NOT) * NCOLS
                for h, n in enumerate(NSPLIT):
                    if not is_act(b, h):
                        continue
                    k = len(NSPLIT) * b + h
                    scalar.wait_ge(s_mm, k + 1)
                    bank = (k % 8) * 512
                    c0 = c0s[h]
                    scalar.copy(ot_sb[:, so + c0:so + c0 + n],
                                ps[0:P, bank:bank + n]).then_inc(s_cs, 1)
                # this ring's outputs issue right after the drains they need
                if OUT_RING.get(b) == 'C':
                    nv, _ = copies_done_before(len(NSPLIT) * (b + 1))
                    scalar.wait_ge(s_cv, nv)
                    out_dma(scalar, b)

        @block.gpsimd
        def _(gpsimd):
            # this ring's outputs ride SWDGE: receipt stalls stay off the
            # input rings, and Q7 is idle anyway
            for b in sorted(k for k, r in OUT_RING.items() if r == 'A'):
                nv, ns = copies_done_before(len(NSPLIT) * (b + 1))
                gpsimd.wait_ge(s_cv, nv)
                gpsimd.wait_ge(s_cs, ns)
                out_dma(gpsimd, b)
            for ring, sem in s_pr.items():
                n = sum(1 for r in OUT_RING.values() if r == ring)
                gpsimd.wait_ge(sem, 16 * n)

        # block exit emits drain + all-engine barrier; then reset the kernel
        # sems to 0 so the NEFF can re-execute
        block_cm.__exit__(None, None, None)
        nums = sorted(s.num for s in
                      [s_tw, s_mm, s_cv, s_cs, s_prA, s_prB, s_prC] + s_ch)
        nc.gpsimd.dma_reset(range(nums[0], nums[-1] + 1))
        nc.gpsimd.sem_clear(range(nums[0], nums[-1] + 1))

    nc.compile()
    return nc


def _get_graph():
    if "nc" not in _CACHE:
        _CACHE["nc"] = _build_graph()
    return _CACHE["nc"]


def _host_prep(x, features, filter_params, W1, b1, W2, b2):
    """Selector MLP + combined filters + layout prep. All tiny or memory-bound."""
    import ml_dtypes
    from numpy.lib.stride_tricks import sliding_window_view
    bf16 = ml_dtypes.bfloat16

    x = np.ascontiguousarray(x, dtype=np.float32)
    # selector MLP (torch Linear convention)
    h = np.maximum(features @ W1.T + b1, 0.0)
    logits = h @ W2.T + b2
    e = np.exp(logits - logits.max(axis=-1, keepdims=True))
    w = e / e.sum(axis=-1, keepdims=True)                      # (B, 8)
    kb = (w @ filter_params[:, 0, :]).astype(np.float32)       # (B, 31)

    # overlapped interleave: X[b, q, c] = x[b, c*98 + q - 15]
    span = (NCOLS - 1) * P + 128
    xp = np.zeros((B, span), dtype=np.float32)
    xp[:, PAD:PAD + L] = x
    win = sliding_window_view(xp, 128, axis=1)                 # (B, span-127, 128)
    xt = win[:, ::P][:, :NCOLS].transpose(0, 2, 1)             # (B, 128, 1338)

    # banded Toeplitz weight: T[q, m] = kb[q - m], 0 <= q-m <= 30
    q = np.arange(128)[:, None]
    m = np.arange(128)[None, :]          # padded to 128 cols for PE FWL
    t_i = q - m
    mask = (t_i >= 0) & (t_i <= 30)
    tw = np.zeros((B, 128, 128), dtype=np.float32)
    tw[:, mask] = kb[:, t_i[mask]]

    def pack(a):  # (B, Pdim, C) -> per-core (Pdim, BPC*C) bf16
        Pd, C = a.shape[1], a.shape[2]
        return [np.ascontiguousarray(
                    a[i * BPC:(i + 1) * BPC].transpose(1, 0, 2).reshape(Pd, BPC * C)
                ).astype(bf16) for i in range(N_CORES)]

    return pack(xt), pack(tw)


def _run(inputs, trace=False, trace_cores=None):
    """Shard, execute on 8 NeuronCores, gather. Returns (y, exec_time_ns)."""
    from concourse.bass_utils import run_bass_kernel_spmd

    xts, tws = _host_prep(**inputs)
    nc = _get_graph()
    in_maps = [{"xt": xts[i], "tw": tws[i]} for i in range(N_CORES)]
    res = run_bass_kernel_spmd(nc, in_maps, core_ids=list(range(N_CORES)),
                               trace=trace, trace_cores=trace_cores)
    # gather: per-core out [P, BPC*NCOLS]; sample block.T.flatten()[:L] -> y[b]
    y = np.empty((B, L), dtype=np.float32)
    for i in range(N_CORES):
        yc = np.asarray(res.results[i]["out"]).astype(np.float32)
        yc = yc.reshape(P, BPC, NCOLS).transpose(1, 2, 0)      # (BPC, NCOLS, P)
        y[i * BPC:(i + 1) * BPC] = yc.reshape(BPC, NCOLS * P)[:, :L]
    return y, res.exec_time_ns


def kernel(x, features, filter_params, W1, b1, W2, b2):
    y, _ = _run(dict(x=x, features=features, filter_params=filter_params,
                     W1=W1, b1=b1, W2=W2, b2=b2))
    return y


# revision 28
# speedup vs baseline: 1.0523x; 1.0150x over previous
"""AdaptiveFilterBank Trainium2 kernel (8 NeuronCores, data-parallel over batch).

Math: reference = conv1d(x, filters) then per-sample softmax-weighted sum over
the 8 filter channels. The weighted sum commutes with the (linear) conv, so
each sample needs ONE length-31 conv with a combined per-sample filter
    kb[b] = softmax(MLP(features[b])) @ filter_params      (tiny, host-side)

Device formulation (overlapped interleave, P=98): per sample lay x out as
    X[q, c] = x[c*98 + q - 15]      (zero-padded), [128, 1338] in SBUF
so each SBUF column holds a 128-wide window covering the 98 outputs of that
column plus the +-15 conv halo. Then the whole 'same' cross-correlation is ONE
matmul per output tile:
    Y[m, c] = sum_q T[q, m] X[q, c],   T[q, m] = kb[q - m]  (0 <= q-m <= 30)
with Y[m, c] = y[c*98 + m], m in [0, 98).

All PE traffic is bf16; accumulation fp32 in PSUM; HBM traffic bf16 both ways.

DMA schedule (v2): profile showed the v1 single-ring schedule left the SDMA
engines idle 52% of the span — every DMA shared the one qSPDynamicHW FIFO, so
each transfer's ~1-2us completion-receipt stall (sem-inc descriptor waits for
the data-write receipt) serialized with the next transfer's data, and output
writes interleaved with the input stream. v2 spreads traffic over all three
DGE rings:
  - SP   HWDGE ring: input chunks 0,2,4,6 (one per even sample)
  - ACT  HWDGE ring: tw (Toeplitz weights) first, then chunks 1,3,5,7
  - Pool SWDGE ring: all output-group DMAs (issued by the otherwise-idle Q7)
Alternating rings lets one ring's receipt stall hide behind the other ring's
data, and keeps output HBM-write receipts off the input stream entirely.
PE additionally runs 6 throwaway matmuls at program start so the HAM clock
gate (cold 1.2 GHz -> warm 2.4 GHz after ~3.4us of activity) is released by
the time the first real chunk lands.

Sharding: batch 64 -> 8 samples per core; filter/MLP params host-computed.
"""

import numpy as np

B = 64
L = 131072
N_CORES = 8
BPC = B // N_CORES          # samples per core
KLEN = 31
PAD = 15
P = 98                      # outputs per interleave column (128 - 30 halo)
NCOLS = 1338                # ceil(L / P) input/output columns per sample
NSPLIT = (512, 512, 314)    # matmul N tiling of the 1338 columns
_CACHE = {}


def _build_graph():
    """Raw Bacc graph with hand-rolled semaphores (Tile's fixed epilogue —
    kernel-tail drain + EVSEM butterfly — measured ~9 us, so we skip Tile)."""
    from concourse import bacc, mybir

    dt = mybir.dt
    nc = bacc.Bacc("TRN2", target_bir_lowering=False, debug=False,
                   num_devices=N_CORES)

    x_ext = nc.dram_tensor("xt", [128, BPC * NCOLS], dt.bfloat16,
                           kind="ExternalInput").ap()
    t_ext = nc.dram_tensor("tw", [128, BPC * 128], dt.bfloat16,
                           kind="ExternalInput").ap()
    out_ext = nc.dram_tensor("out", [P, BPC * NCOLS], dt.bfloat16,
                             kind="ExternalOutput").ap()

    NOT = 8                 # output staging slots (1 per sample: no WAR waits)
    c0s = [sum(NSPLIT[:h]) for h in range(len(NSPLIT))]
    # engine that copies tile (b, h): ACT the middle tile of every sample;
    # for the LAST sample ACT takes h0+h1 and DVE only the short h2, so the
    # final drain (which gates the tail output DMA) lands ~1.7us sooner
    def is_act(b, h):
        if b == BPC - 1:
            return h in (0, 1)
        return h == 1

    def copies_done_before(k):
        """(#DVE, #ACT) copies among global tiles 0..k-1."""
        nv = sum(1 for j in range(k)
                 if not is_act(j // len(NSPLIT), j % len(NSPLIT)))
        ns = k - nv
        return nv, ns

    from contextlib import ExitStack
    stack = ExitStack()
    with (
        nc.sbuf_tensor("xt_sb", [128, BPC * NCOLS], dt.bfloat16) as xt_sb,
        nc.sbuf_tensor("tw_sb", [128, BPC * 128], dt.bfloat16) as tw_sb,
        nc.sbuf_tensor("ot_sb", [P, NOT * NCOLS], dt.bfloat16) as ot_sb,
        nc.psum_tensor("ps", [128, 8 * 512], dt.float32) as ps,
        nc.semaphore("s_tw") as s_tw,
        nc.semaphore("s_mm") as s_mm,
        nc.semaphore("s_cv") as s_cv,
        nc.semaphore("s_cs") as s_cs,
        stack,
    ):
        # one sem per input chunk: a sem shared by several DMAs can't prove
        # WHICH one finished (engines complete out of lockstep)
        s_ch = [stack.enter_context(nc.semaphore(f"s_ch{i}"))
                for i in range(BPC)]
        # outputs: per-sample DMAs spread over ALL THREE rings (one output
        # issues per ~2.3us per ring — data + HBM-write receipt — so one or
        # two rings can't keep up with the ~1.1us/sample drain cadence). One
        # shared sem per ring: a partial count can't prove a PREFIX of DMAs
        # done, but the FULL sum (16 per DMA) does prove all of them landed.
        OUT_RING = {0: 'A', 2: 'A',             # Pool SWDGE (slow ~80 GB/s:
                                                # early samples only)
                    1: 'B', 4: 'B', 6: 'B',     # SP  HWDGE
                    3: 'C', 5: 'C', 7: 'C'}     # ACT HWDGE (its own drains
                                                # gate the tail sample anyway)
        s_prA = stack.enter_context(nc.semaphore("s_prA"))
        s_prB = stack.enter_context(nc.semaphore("s_prB"))
        s_prC = stack.enter_context(nc.semaphore("s_prC"))
        s_pr = {'A': s_prA, 'B': s_prB, 'C': s_prC}
        block_cm = nc.Block(no_gpsimd_drain=True)
        block = block_cm.__enter__()

        def out_dma(eng, b):
            so = (b % NOT) * NCOLS
            eng.dma_start(out=out_ext[:, b * NCOLS:(b + 1) * NCOLS],
                          in_=ot_sb[:, so:so + NCOLS]
                          ).then_inc(s_pr[OUT_RING[b]], 16)

        @block.sync
        def _(sync):
            # weights first (gate every matmul), then even chunks; the SP
            # ring's first packet hits ~1.4us after dispatch vs ~2.8us on the
            # ACT ring (ACT_TABLE_LOAD runs there first)
            sync.dma_start(out=tw_sb[:], in_=t_ext[:]).then_inc(s_tw, 16)
            for b in range(0, BPC, 2):
                sync.dma_start(
                    out=xt_sb[:, b * NCOLS:(b + 1) * NCOLS],
                    in_=x_ext[:, b * NCOLS:(b + 1) * NCOLS],
                ).then_inc(s_ch[b], 16)
            for b in sorted(k for k, r in OUT_RING.items() if r == 'B'):
                nv, ns = copies_done_before(len(NSPLIT) * (b + 1))
                sync.wait_ge(s_cv, nv)
                sync.wait_ge(s_cs, ns)
                out_dma(sync, b)

        @block.tensor
        def _(tensor):
            # HAM pre-warm: garbage matmuls into bank 7 (overwritten later by
            # real tile k=7 whose readers gate on s_mm) while inputs stream in
            for _ in range(6):
                tensor.matmul(ps[:, 7 * 512:7 * 512 + 512],
                              tw_sb[:, 0:128], xt_sb[:, 0:512],
                              start=True, stop=True)
            tensor.wait_ge(s_tw, 16)
            for b in range(BPC):
                tensor.wait_ge(s_ch[b], 16)
                for h, n in enumerate(NSPLIT):
                    k = len(NSPLIT) * b + h
                    if k >= 8:
                        # recycled bank is drained by exactly ONE engine
                        nv, ns = copies_done_before(k - 7)
                        if is_act((k - 8) // len(NSPLIT), (k - 8) % len(NSPLIT)):
                            tensor.wait_ge(s_cs, ns)
                        else:
                            tensor.wait_ge(s_cv, nv)
                    bank = (k % 8) * 512
                    c0 = c0s[h]
                    tensor.matmul(
                        ps[:, bank:bank + n],
                        tw_sb[:, b * 128:(b + 1) * 128],
                        xt_sb[:, b * NCOLS + c0:b * NCOLS + c0 + n],
                        start=True, stop=True).then_inc(s_mm, 1)

        @block.vector
        def _(vector):
            for b in range(BPC):
                so = (b % NOT) * NCOLS
                for h, n in enumerate(NSPLIT):
                    if is_act(b, h):
                        continue
                    k = len(NSPLIT) * b + h
                    vector.wait_ge(s_mm, k + 1)
                    bank = (k % 8) * 512
                    c0 = c0s[h]
                    vector.tensor_copy(ot_sb[:, so + c0:so + c0 + n],
                                       ps[0:P, bank:bank + n]).then_inc(s_cv, 1)

        @block.scalar
        def _(scalar):
            # odd chunks on the ACT HWDGE ring
            for b in range(1, BPC, 2):
                scalar.dma_start(
                    out=xt_sb[:, b * NCOLS:(b + 1) * NCOLS],
                    in_=x_ext[:, b * NCOLS:(b + 1) * NCOLS],
                ).then_inc(s_ch[b], 16)
            for b in range(BPC):
                so = (b % NOT) * NCOLS
                for h, n in enumerate(NSPLIT):
                    if not is_act(b, h):
                        continue
                    k = len(NSPLIT) * b + h
                    scalar.wait_ge(s_mm, k + 1)
                    bank = (k % 8) * 512
                    c0 = c0s[h]
                    scalar.copy(ot_sb[:, so + c0:so + c0 + n],
                                ps[0:P, bank:bank + n]).then_inc(s_cs, 1)
                # this ring's outputs issue right after the drains they need
                if OUT_RING.get(b) == 'C':
                    nv, _ = copies_done_before(len(NSPLIT) * (b + 1))
                    scalar.wait_ge(s_cv, nv)
                    out_dma(scalar, b)

        @block.gpsimd
        def _(gpsimd):
            # this ring's outputs ride SWDGE: receipt stalls stay off the
            # input rings, and Q7 is idle anyway
            for b in sorted(k for k, r in OUT_RING.items() if r == 'A'):
                nv, ns = copies_done_before(len(NSPLIT) * (b + 1))
                gpsimd.wait_ge(s_cv, nv)
                gpsimd.wait_ge(s_cs, ns)
                out_dma(gpsimd, b)
            for ring, sem in s_pr.items():
                n = sum(1 for r in OUT_RING.values() if r == ring)
                gpsimd.wait_ge(sem, 16 * n)

        # block exit emits drain + all-engine barrier; then reset the kernel
        # sems to 0 so the NEFF can re-execute
        block_cm.__exit__(None, None, None)
        nums = sorted(s.num for s in
                      [s_tw, s_mm, s_cv, s_cs, s_prA, s_prB, s_prC] + s_ch)
        nc.gpsimd.dma_reset(range(nums[0], nums[-1] + 1))
        nc.gpsimd.sem_clear(range(nums[0], nums[-1] + 1))

    nc.compile()
    return nc


def _get_graph():
    if "nc" not in _CACHE:
        _CACHE["nc"] = _build_graph()
    return _CACHE["nc"]


def _host_prep(x, features, filter_params, W1, b1, W2, b2):
    """Selector MLP + combined filters + layout prep. All tiny or memory-bound."""
    import ml_dtypes
    from numpy.lib.stride_tricks import sliding_window_view
    bf16 = ml_dtypes.bfloat16

    x = np.ascontiguousarray(x, dtype=np.float32)
    # selector MLP (torch Linear convention)
    h = np.maximum(features @ W1.T + b1, 0.0)
    logits = h @ W2.T + b2
    e = np.exp(logits - logits.max(axis=-1, keepdims=True))
    w = e / e.sum(axis=-1, keepdims=True)                      # (B, 8)
    kb = (w @ filter_params[:, 0, :]).astype(np.float32)       # (B, 31)

    # overlapped interleave: X[b, q, c] = x[b, c*98 + q - 15]
    span = (NCOLS - 1) * P + 128
    xp = np.zeros((B, span), dtype=np.float32)
    xp[:, PAD:PAD + L] = x
    win = sliding_window_view(xp, 128, axis=1)                 # (B, span-127, 128)
    xt = win[:, ::P][:, :NCOLS].transpose(0, 2, 1)             # (B, 128, 1338)

    # banded Toeplitz weight: T[q, m] = kb[q - m], 0 <= q-m <= 30
    q = np.arange(128)[:, None]
    m = np.arange(128)[None, :]          # padded to 128 cols for PE FWL
    t_i = q - m
    mask = (t_i >= 0) & (t_i <= 30)
    tw = np.zeros((B, 128, 128), dtype=np.float32)
    tw[:, mask] = kb[:, t_i[mask]]

    def pack(a):  # (B, Pdim, C) -> per-core (Pdim, BPC*C) bf16
        Pd, C = a.shape[1], a.shape[2]
        return [np.ascontiguousarray(
                    a[i * BPC:(i + 1) * BPC].transpose(1, 0, 2).reshape(Pd, BPC * C)
                ).astype(bf16) for i in range(N_CORES)]

    return pack(xt), pack(tw)


def _run(inputs, trace=False, trace_cores=None):
    """Shard, execute on 8 NeuronCores, gather. Returns (y, exec_time_ns)."""
    from concourse.bass_utils import run_bass_kernel_spmd

    xts, tws = _host_prep(**inputs)
    nc = _get_graph()
    in_maps = [{"xt": xts[i], "tw": tws[i]} for i in range(N_CORES)]
    res = run_bass_kernel_spmd(nc, in_maps, core_ids=list(range(N_CORES)),
                               trace=trace, trace_cores=trace_cores)
    # gather: per-core out [P, BPC*NCOLS]; sample block.T.flatten()[:L] -> y[b]
    y = np.empty((B, L), dtype=np.float32)
    for i in range(N_CORES):
        yc = np.asarray(res.results[i]["out"]).astype(np.float32)
        yc = yc.reshape(P, BPC, NCOLS).transpose(1, 2, 0)      # (BPC, NCOLS, P)
        y[i * BPC:(i + 1) * BPC] = yc.reshape(BPC, NCOLS * P)[:, :L]
    return y, res.exec_time_ns


def kernel(x, features, filter_params, W1, b1, W2, b2):
    y, _ = _run(dict(x=x, features=features, filter_params=filter_params,
                     W1=W1, b1=b1, W2=W2, b2=b2))
    return y
